# revision 3
# baseline (speedup 1.0000x reference)
"""CorrespondenceGeneration kernel for 8 TRN2 NeuronCores.

Reference computation (per item): unit-normalize features over channels,
build 3x3 patch matrices, corr = inp_patches^T @ ref_patches, argmax over
ref patches (first occurrence on ties), convert argmax index to flow,
9 tensor-shifts, channel reorder.

Sharding (ref-axis split, mirroring the chunked running-argmax in the
original code): core q -> (item = q//4, ref-column strip = q%4 of 2224
cols).  Each core computes ALL 8836 corr rows (70 blocks of 128) against
its 2224 ref columns in 5 column chunks [512,512,512,416,272], each chunk
accumulated over 5 K-passes into its own PSUM bank.  The DVE drains each
bank directly with pool_max (window 16) into per-block segment maxes; a
per-block MAX8 + FIND_INDEX8 over the 139 segment maxes yields top-8
(value, segment) pairs.  The host combines the 4 strips with a running
max (earliest strip/segment wins ties), exactly rescoring the winning
16-col segment, and fully rescoring near-tie rows -- identical tie/error
semantics to the fp32r margin scheme validated before.

vs the previous inp-split kernel this removes the PSUM->SBUF scalar
copies (163us) and the second full-width DVE scan (182us), cuts PE
padding (70x5x2224 = 778k cycles vs 18x5x8960 = 806k), and shrinks the
DMA-gated head (first matmul needs only 576KB) and the drain tail.

Note: the reference's per-patch-column normalization of ref divides every
column by ||col||+eps with ||col|| == 3 exactly (9 unit-norm pixels), a
global positive scale that argmax is invariant to -- so it is skipped.
"""

import sys

if "/opt/trn_rl_repo" not in sys.path:
    sys.path.insert(0, "/opt/trn_rl_repo")

import numpy as np

# ---- problem constants (hardcoded; kernel.py must be self-contained) ----
N_ITEMS = 2
C = 64
H = W = 96
PS = 3
HP = WP = H - PS + 1          # 94
NPATCH = HP * WP              # 8836
K = C * PS * PS               # 576
KPAD = 640                    # 5 x 128
KCH = 5                       # K chunks of 128
N_CORES = 8
STRIPS = 4                    # ref-column strips per item
STRIP = 2224                  # 139 x 16; 4*2224 = 8896 >= 8836
NBLK = 70                     # row blocks of 128 (8960 >= 8836)
# column chunks within a strip: each <= 512 (one PSUM bank), >= 256
# (fp32r fast path), multiple of WINDOW
CHUNK_W = (512, 512, 512, 416, 272)
CHUNK_OFF = (0, 512, 1024, 1536, 1952)
N_CHUNKS = len(CHUNK_W)
WINDOW = 16                   # pool_max window
NSEG_C = tuple(w // WINDOW for w in CHUNK_W)     # (32,32,32,26,17)
SEG_OFF = (0, 32, 64, 96, 122)
NSEG = 139                    # segments per strip per block
SM_PAD = 144
GSIZE = 14                    # row blocks per pipeline group
N_GROUPS = NBLK // GSIZE      # 5
EPS_NORMALIZE = 1e-12

# matmul input dtype: float32r (1 cyc/row); max error ~2.6e-4 measured.
MM_DTYPE = "float32r"
# rows whose top1-top2 segment-max gap is below this get an exact host
# rescore over the full ref row (~hundreds of rows).
MARGIN_THRESH = 4e-3

_COMPILED = {}


def _build_module():
    import concourse.bacc as bacc
    from concourse.tile import TileContext
    from concourse import mybir

    dt_mm = getattr(mybir.dt, MM_DTYPE)
    f32 = mybir.dt.float32
    nc = bacc.Bacc("TRN2", target_bir_lowering=False, debug=False,
                   num_devices=N_CORES)
    inp_d = nc.dram_tensor("inp", [NBLK, 128, KCH * 128], dt_mm,
                           kind="ExternalInput").ap()
    ref_d = nc.dram_tensor("ref", [KCH, 128, STRIP], dt_mm,
                           kind="ExternalInput").ap()
    val_d = nc.dram_tensor("val", [128, NBLK * 8], f32,
                           kind="ExternalOutput").ap()
    idx_d = nc.dram_tensor("idx", [128, NBLK * 8], mybir.dt.uint32,
                           kind="ExternalOutput").ap()

    with TileContext(nc) as tc:
        with tc.tile_pool(name="ref", bufs=1) as ref_pool, \
             tc.tile_pool(name="inp", bufs=2 * GSIZE) as inp_pool, \
             tc.tile_pool(name="sm", bufs=2 * GSIZE) as sm_pool, \
             tc.tile_pool(name="acc", bufs=1) as acc_pool, \
             tc.tile_pool(name="psum", bufs=8, space="PSUM") as psum_pool:
            ref_sb = ref_pool.tile([128, KCH * STRIP], dt_mm, name="ref_sb")
            acc_val = acc_pool.tile([128, NBLK * 8], f32, name="acc_val")
            acc_idx = acc_pool.tile([128, NBLK * 8], mybir.dt.uint32,
                                    name="acc_idx")
            inp_tiles = [None] * NBLK
            sm_tiles = [None] * NBLK

            def emit_inp_dma(b):
                t = inp_pool.tile([128, KCH * 128], dt_mm, tag="inp",
                                  name=f"inp_{b}")
                nc.sync.dma_start(t, inp_d[b])
                inp_tiles[b] = t

            def emit_ref_dma(c, ks):
                off, w = CHUNK_OFF[c], CHUNK_W[c]
                for k in ks:
                    nc.sync.dma_start(
                        ref_sb[:, k * STRIP + off:k * STRIP + off + w],
                        ref_d[k, :, off:off + w])

            # startup DMA order: gate the first matmuls on as little data
            # as possible (ref c0/k0 piece + inp block 0 = 576KB), then
            # interleave remaining ref chunk-columns with inp blocks in
            # roughly first-use order.
            emit_ref_dma(0, [0])
            emit_inp_dma(0)
            emit_ref_dma(0, range(1, KCH))
            emit_inp_dma(1)
            emit_ref_dma(1, range(KCH))
            emit_inp_dma(2)
            emit_inp_dma(3)
            emit_ref_dma(2, range(KCH))
            emit_inp_dma(4)
            emit_inp_dma(5)
            emit_ref_dma(3, range(KCH))
            for b in range(6, 9):
                emit_inp_dma(b)
            emit_ref_dma(4, range(KCH))
            for b in range(9, GSIZE):
                emit_inp_dma(b)

            for g in range(N_GROUPS):
                if g + 1 < N_GROUPS:
                    for b in range((g + 1) * GSIZE, (g + 2) * GSIZE):
                        emit_inp_dma(b)
                for ci in range(N_CHUNKS):
                    off, w = CHUNK_OFF[ci], CHUNK_W[ci]
                    ns, so = NSEG_C[ci], SEG_OFF[ci]
                    for b in range(g * GSIZE, (g + 1) * GSIZE):
                        if ci == 0:
                            sm_tiles[b] = sm_pool.tile(
                                [128, SM_PAD], f32, tag="sm", name=f"sm_{b}")
                        pt = psum_pool.tile([128, 512], f32, tag="ps",
                                            name=f"pt_{ci}_{b}")
                        for k in range(KCH):
                            nc.tensor.matmul(
                                pt[:, :w],
                                inp_tiles[b][:, k * 128:(k + 1) * 128],
                                ref_sb[:, k * STRIP + off:
                                       k * STRIP + off + w],
                                start=(k == 0), stop=(k == KCH - 1))
                        nc.vector.tensor_reduce(
                            sm_tiles[b][:, so:so + ns],
                            pt[:, :w].rearrange("p (s w) -> p s w",
                                                w=WINDOW),
                            mybir.AxisListType.X, mybir.AluOpType.max)
                        if ci == N_CHUNKS - 1:
                            s8 = b * 8
                            nc.vector.max(acc_val[:, s8:s8 + 8],
                                          sm_tiles[b][:, :NSEG])
                            nc.vector.max_index(acc_idx[:, s8:s8 + 8],
                                                acc_val[:, s8:s8 + 8],
                                                sm_tiles[b][:, :NSEG])
                lo, hi = g * GSIZE * 8, (g + 1) * GSIZE * 8
                nc.sync.dma_start(val_d[:, lo:hi], acc_val[:, lo:hi])
                nc.sync.dma_start(idx_d[:, lo:hi], acc_idx[:, lo:hi])

    nc.compile()
    return nc


def _get_nc():
    if "nc" not in _COMPILED:
        _COMPILED["nc"] = _build_module()
    return _COMPILED["nc"]


def _unit_channels(f):
    # f: (N, C, H, W) float32; unit L2 norm over channels per pixel
    n = np.sqrt(np.sum(f * f, axis=1, keepdims=True, dtype=np.float32))
    return (f / np.maximum(n, EPS_NORMALIZE)).astype(np.float32)


def _patches(f):
    # f: (C, H, W) -> (K, NPATCH), row index = c*9 + dy*3 + dx
    out = np.empty((C, PS * PS, HP, WP), np.float32)
    for dy in range(PS):
        for dx in range(PS):
            out[:, dy * PS + dx] = f[:, dy:dy + HP, dx:dx + WP]
    return out.reshape(K, NPATCH)


def _prep_inputs(dense_features1, dense_features2):
    fi = _unit_channels(np.ascontiguousarray(dense_features1, np.float32))
    fr = _unit_channels(np.ascontiguousarray(dense_features2, np.float32))
    in_maps = []
    mats = []
    for n in range(N_ITEMS):
        inp_full = _patches(fi[n])                       # (576, 8836)
        ref_full = _patches(fr[n])                       # (576, 8836)
        mats.append((inp_full, ref_full))
        # inp blocks: [b, k_partition, kc*128 + m] so each block is one
        # contiguous 320KB DMA with 2560B partition lines
        P = np.zeros((KPAD, NBLK * 128), np.float32)
        P[:K, :NPATCH] = inp_full
        inp_dev = np.ascontiguousarray(
            P.reshape(KCH, 128, NBLK, 128)
            .transpose(2, 1, 0, 3).reshape(NBLK, 128, KCH * 128))
        R = np.zeros((KPAD, STRIPS * STRIP), np.float32)
        R[:K, :NPATCH] = ref_full
        R = R.reshape(KCH, 128, STRIPS * STRIP)
        for s in range(STRIPS):
            ref_dev = np.ascontiguousarray(R[:, :, s * STRIP:(s + 1) * STRIP])
            in_maps.append({"inp": inp_dev, "ref": ref_dev})
    return in_maps, mats


def _combine_item(res_list, inp_full, ref_full):
    # res_list: 4 strip dicts with val (128, 560) f32, idx (128, 560) u32.
    # slot layout: block b at [:, b*8:(b+1)*8]; corr row = b*128 + p.
    V = np.stack([
        r["val"].reshape(128, NBLK, 8).transpose(1, 0, 2)
        .reshape(NBLK * 128, 8)[:NPATCH] for r in res_list], axis=1)
    S = np.stack([
        r["idx"].reshape(128, NBLK, 8).transpose(1, 0, 2)
        .reshape(NBLK * 128, 8)[:NPATCH] for r in res_list],
        axis=1).astype(np.int64)
    base = S * WINDOW + (np.arange(STRIPS, dtype=np.int64)
                         * STRIP)[None, :, None]
    V32 = V.reshape(NPATCH, STRIPS * 8)
    B32 = base.reshape(NPATCH, STRIPS * 8)
    rows = np.arange(NPATCH)
    v1i = np.argmax(V32, axis=1)
    v1 = V32[rows, v1i]
    V32m = V32.copy()
    V32m[rows, v1i] = -np.inf
    margin = v1 - V32m.max(axis=1)
    winbase = B32[rows, v1i]

    flagged = (margin < MARGIN_THRESH) | (winbase >= NPATCH)
    max_idx = np.empty(NPATCH, np.int64)

    # exact rescore of each row's winning 16-col segment, grouped by
    # segment so each group is one small sgemm
    order = np.argsort(winbase, kind="stable")
    ob = winbase[order]
    bounds = np.flatnonzero(np.diff(ob)) + 1
    for grp in np.split(order, bounds):
        b0 = winbase[grp[0]]
        if b0 >= NPATCH:
            continue
        cols = np.arange(b0, min(b0 + WINDOW, NPATCH))
        sc = inp_full[:, grp].T @ ref_full[:, cols]
        max_idx[grp] = cols[np.argmax(sc, axis=1)]
        if cols.size >= 2:
            t2 = np.partition(sc, cols.size - 2, axis=1)[:, -2:]
            risky = np.flatnonzero(t2[:, 1] - t2[:, 0] < 1e-3)
            if risky.size:
                sc64 = inp_full[:, grp[risky]].T.astype(np.float64) @ \
                    ref_full[:, cols].astype(np.float64)
                max_idx[grp[risky]] = cols[np.argmax(sc64, axis=1)]

    fl = np.flatnonzero(flagged)
    if fl.size:
        # exact rescore of near-tie rows over the full ref row: fp32 sgemm
        # first, fp64 only for rows still ambiguous at fp32 rounding scale
        corr = inp_full[:, fl].T @ ref_full
        max_idx[fl] = np.argmax(corr, axis=1)
        top2 = np.partition(corr, corr.shape[1] - 2, axis=1)[:, -2:]
        risky = np.flatnonzero(top2[:, 1] - top2[:, 0] < 1e-3)
        if risky.size:
            corr64 = inp_full[:, fl[risky]].T.astype(np.float64) @ \
                ref_full.astype(np.float64)
            max_idx[fl[risky]] = np.argmax(corr64, axis=1)
    return max_idx


def _flow_output(max_idx):
    # max_idx: (NPATCH,) int -> (18, H, W) float32, mirroring the reference
    mi = max_idx.reshape(HP, WP)
    fw = (mi % WP).astype(np.float32) - np.arange(WP, dtype=np.float32)[None, :]
    fh = (mi // WP).astype(np.float32) - np.arange(HP, dtype=np.float32)[:, None]
    flow = np.stack([fw, fh], axis=-1)                     # (94, 94, 2)
    flow = np.pad(flow, ((0, PS - 1), (0, PS - 1), (0, 0)))  # (96, 96, 2)
    shifted = np.stack([np.pad(flow, ((i, 0), (j, 0), (0, 0)))[:H, :W]
                        for i in range(PS) for j in range(PS)], axis=0)
    out = np.stack([shifted[..., 1], shifted[..., 0]], axis=1)  # (9, 2, H, W)
    return out.reshape(2 * PS * PS, H, W).astype(np.float32)


def kernel(dense_features1, dense_features2):
    from concourse import bass_utils

    nc = _get_nc()
    in_maps, mats = _prep_inputs(dense_features1, dense_features2)
    res = bass_utils.run_bass_kernel_spmd(
        nc, in_maps, core_ids=list(range(N_CORES)))
    out = np.empty((N_ITEMS, 2 * PS * PS, H, W), np.float32)
    for n in range(N_ITEMS):
        max_idx = _combine_item(
            [res.results[n * STRIPS + s] for s in range(STRIPS)], *mats[n])
        out[n] = _flow_output(max_idx)
    return out


# revision 6
# speedup vs baseline: 1.0201x; 1.0201x over previous
"""CorrespondenceGeneration kernel for 8 TRN2 NeuronCores.

Reference computation (per item): unit-normalize features over channels,
build 3x3 patch matrices, corr = inp_patches^T @ ref_patches, argmax over
ref patches (first occurrence on ties), convert argmax index to flow,
9 tensor-shifts, channel reorder.

Sharding (ref-axis split, mirroring the chunked running-argmax in the
original code): core q -> (item = q//4, ref-column strip = q%4 of 2224
cols).  Each core computes ALL 8836 corr rows (70 blocks of 128) against
its 2224 ref columns in 5 column chunks [512,512,512,416,272], each chunk
accumulated over 5 K-passes into its own PSUM bank.  The DVE drains each
bank directly with pool_max (window 16) into per-block segment maxes; a
per-block MAX8 + FIND_INDEX8 over the 139 segment maxes yields top-8
(value, segment) pairs.  The host combines the 4 strips with a running
max (earliest strip/segment wins ties), exactly rescoring the winning
16-col segment, and fully rescoring near-tie rows -- identical tie/error
semantics to the fp32r margin scheme validated before.

vs the previous inp-split kernel this removes the PSUM->SBUF scalar
copies (163us) and the second full-width DVE scan (182us), cuts PE
padding (70x5x2224 = 778k cycles vs 18x5x8960 = 806k), and shrinks the
DMA-gated head (first matmul needs only 576KB) and the drain tail.

Note: the reference's per-patch-column normalization of ref divides every
column by ||col||+eps with ||col|| == 3 exactly (9 unit-norm pixels), a
global positive scale that argmax is invariant to -- so it is skipped.
"""

import sys

if "/opt/trn_rl_repo" not in sys.path:
    sys.path.insert(0, "/opt/trn_rl_repo")

import numpy as np

# ---- problem constants (hardcoded; kernel.py must be self-contained) ----
N_ITEMS = 2
C = 64
H = W = 96
PS = 3
HP = WP = H - PS + 1          # 94
NPATCH = HP * WP              # 8836
K = C * PS * PS               # 576
KPAD = 640                    # 5 x 128
KCH = 5                       # K chunks of 128
N_CORES = 8
STRIPS = 4                    # ref-column strips per item
STRIP = 2224                  # 139 x 16; 4*2224 = 8896 >= 8836
NBLK = 70                     # row blocks of 128 (8960 >= 8836)
# column chunks within a strip: each <= 512 (one PSUM bank), >= 256
# (fp32r fast path), multiple of WINDOW.  Measured issue rate is
# max(w/2.2GHz, ~140ns) per matmul, so one ~304-wide chunk rides the
# fixed per-instruction floor for free.
CHUNK_W = (512, 512, 512, 384, 304)
CHUNK_OFF = (0, 512, 1024, 1536, 1920)
N_CHUNKS = len(CHUNK_W)
WINDOW = 16                   # segment-max window
NSEG_C = tuple(w // WINDOW for w in CHUNK_W)     # (32,32,32,24,19)
SEG_OFF = (0, 32, 64, 96, 120)
NSEG = 139                    # segments per strip per block
SM_PAD = 144
GSIZE = 14                    # row blocks per pipeline group
N_GROUPS = NBLK // GSIZE      # 5
EPS_NORMALIZE = 1e-12

# matmul input dtype: float32r (1 cyc/row); max error ~2.6e-4 measured.
MM_DTYPE = "float32r"
# rows whose top1-top2 segment-max gap is below this get an exact host
# rescore over the full ref row (~hundreds of rows).
MARGIN_THRESH = 4e-3

_COMPILED = {}


def _build_module():
    import concourse.bacc as bacc
    from concourse.tile import TileContext
    from concourse import mybir

    dt_mm = getattr(mybir.dt, MM_DTYPE)
    f32 = mybir.dt.float32
    nc = bacc.Bacc("TRN2", target_bir_lowering=False, debug=False,
                   num_devices=N_CORES)
    inp_d = nc.dram_tensor("inp", [NBLK, 128, KCH * 128], dt_mm,
                           kind="ExternalInput").ap()
    ref_d = nc.dram_tensor("ref", [KCH, 128, STRIP], dt_mm,
                           kind="ExternalInput").ap()
    val_d = nc.dram_tensor("val", [128, NBLK * 8], f32,
                           kind="ExternalOutput").ap()
    idx_d = nc.dram_tensor("idx", [128, NBLK * 8], mybir.dt.uint32,
                           kind="ExternalOutput").ap()

    with TileContext(nc) as tc:
        with tc.tile_pool(name="ref", bufs=1) as ref_pool, \
             tc.tile_pool(name="inp", bufs=2 * GSIZE) as inp_pool, \
             tc.tile_pool(name="sm", bufs=2 * GSIZE) as sm_pool, \
             tc.tile_pool(name="acc", bufs=1) as acc_pool, \
             tc.tile_pool(name="psum", bufs=8, space="PSUM") as psum_pool:
            ref_sb = ref_pool.tile([128, KCH * STRIP], dt_mm, name="ref_sb")
            acc_val = acc_pool.tile([128, NBLK * 8], f32, name="acc_val")
            acc_idx = acc_pool.tile([128, NBLK * 8], mybir.dt.uint32,
                                    name="acc_idx")
            inp_tiles = [None] * NBLK
            sm_tiles = [None] * NBLK

            def emit_inp_dma(b, split=False):
                t = inp_pool.tile([128, KCH * 128], dt_mm, tag="inp",
                                  name=f"inp_{b}")
                if split:
                    # per-k pieces so the first matmul gates on 64KB
                    for k in range(KCH):
                        nc.sync.dma_start(t[:, k * 128:(k + 1) * 128],
                                          inp_d[b, :, k * 128:(k + 1) * 128])
                else:
                    nc.sync.dma_start(t, inp_d[b])
                inp_tiles[b] = t

            def emit_ref_dma(c, ks):
                off, w = CHUNK_OFF[c], CHUNK_W[c]
                for k in ks:
                    nc.gpsimd.dma_start(
                        ref_sb[:, k * STRIP + off:k * STRIP + off + w],
                        ref_d[k, :, off:off + w])

            # inp blocks stream on the sync queue, ref chunk-columns on the
            # (otherwise idle) gpsimd queue, outputs on the scalar queue --
            # parallel descriptor issue so the first matmul gates on
            # ~320KB and the chunk columns arrive well ahead of the sweep.
            emit_inp_dma(0, split=True)
            for c in range(N_CHUNKS):
                emit_ref_dma(c, range(KCH))
            for b in range(1, GSIZE):
                emit_inp_dma(b)

            for g in range(N_GROUPS):
                if g + 1 < N_GROUPS:
                    for b in range((g + 1) * GSIZE, (g + 2) * GSIZE):
                        emit_inp_dma(b)
                for ci in range(N_CHUNKS):
                    off, w = CHUNK_OFF[ci], CHUNK_W[ci]
                    ns, so = NSEG_C[ci], SEG_OFF[ci]
                    for b in range(g * GSIZE, (g + 1) * GSIZE):
                        if ci == 0:
                            sm_tiles[b] = sm_pool.tile(
                                [128, SM_PAD], f32, tag="sm", name=f"sm_{b}")
                        pt = psum_pool.tile([128, 512], f32, tag="ps",
                                            name=f"pt_{ci}_{b}")
                        for k in range(KCH):
                            nc.tensor.matmul(
                                pt[:, :w],
                                inp_tiles[b][:, k * 128:(k + 1) * 128],
                                ref_sb[:, k * STRIP + off:
                                       k * STRIP + off + w],
                                start=(k == 0), stop=(k == KCH - 1))
                        nc.vector.tensor_reduce(
                            sm_tiles[b][:, so:so + ns],
                            pt[:, :w].rearrange("p (s w) -> p s w",
                                                w=WINDOW),
                            mybir.AxisListType.X, mybir.AluOpType.max)
                        if ci == N_CHUNKS - 1:
                            s8 = b * 8
                            nc.vector.max(acc_val[:, s8:s8 + 8],
                                          sm_tiles[b][:, :NSEG])
                            nc.vector.max_index(acc_idx[:, s8:s8 + 8],
                                                acc_val[:, s8:s8 + 8],
                                                sm_tiles[b][:, :NSEG])
                lo, hi = g * GSIZE * 8, (g + 1) * GSIZE * 8
                nc.scalar.dma_start(val_d[:, lo:hi], acc_val[:, lo:hi])
                nc.scalar.dma_start(idx_d[:, lo:hi], acc_idx[:, lo:hi])

    nc.compile()
    return nc


def _get_nc():
    if "nc" not in _COMPILED:
        _COMPILED["nc"] = _build_module()
    return _COMPILED["nc"]


def _unit_channels(f):
    # f: (N, C, H, W) float32; unit L2 norm over channels per pixel
    n = np.sqrt(np.sum(f * f, axis=1, keepdims=True, dtype=np.float32))
    return (f / np.maximum(n, EPS_NORMALIZE)).astype(np.float32)


def _patches(f):
    # f: (C, H, W) -> (K, NPATCH), row index = c*9 + dy*3 + dx
    out = np.empty((C, PS * PS, HP, WP), np.float32)
    for dy in range(PS):
        for dx in range(PS):
            out[:, dy * PS + dx] = f[:, dy:dy + HP, dx:dx + WP]
    return out.reshape(K, NPATCH)


def _prep_inputs(dense_features1, dense_features2):
    fi = _unit_channels(np.ascontiguousarray(dense_features1, np.float32))
    fr = _unit_channels(np.ascontiguousarray(dense_features2, np.float32))
    in_maps = []
    mats = []
    for n in range(N_ITEMS):
        inp_full = _patches(fi[n])                       # (576, 8836)
        ref_full = _patches(fr[n])                       # (576, 8836)
        mats.append((inp_full, ref_full))
        # inp blocks: [b, k_partition, kc*128 + m] so each block is one
        # contiguous 320KB DMA with 2560B partition lines
        P = np.zeros((KPAD, NBLK * 128), np.float32)
        P[:K, :NPATCH] = inp_full
        inp_dev = np.ascontiguousarray(
            P.reshape(KCH, 128, NBLK, 128)
            .transpose(2, 1, 0, 3).reshape(NBLK, 128, KCH * 128))
        R = np.zeros((KPAD, STRIPS * STRIP), np.float32)
        R[:K, :NPATCH] = ref_full
        R = R.reshape(KCH, 128, STRIPS * STRIP)
        for s in range(STRIPS):
            ref_dev = np.ascontiguousarray(R[:, :, s * STRIP:(s + 1) * STRIP])
            in_maps.append({"inp": inp_dev, "ref": ref_dev})
    return in_maps, mats


def _combine_item(res_list, inp_full, ref_full):
    # res_list: 4 strip dicts with val (128, 560) f32, idx (128, 560) u32.
    # slot layout: block b at [:, b*8:(b+1)*8]; corr row = b*128 + p.
    V = np.stack([
        r["val"].reshape(128, NBLK, 8).transpose(1, 0, 2)
        .reshape(NBLK * 128, 8)[:NPATCH] for r in res_list], axis=1)
    S = np.stack([
        r["idx"].reshape(128, NBLK, 8).transpose(1, 0, 2)
        .reshape(NBLK * 128, 8)[:NPATCH] for r in res_list],
        axis=1).astype(np.int64)
    base = S * WINDOW + (np.arange(STRIPS, dtype=np.int64)
                         * STRIP)[None, :, None]
    V32 = V.reshape(NPATCH, STRIPS * 8)
    B32 = base.reshape(NPATCH, STRIPS * 8)
    rows = np.arange(NPATCH)
    v1i = np.argmax(V32, axis=1)
    v1 = V32[rows, v1i]
    V32m = V32.copy()
    V32m[rows, v1i] = -np.inf
    margin = v1 - V32m.max(axis=1)
    winbase = B32[rows, v1i]

    flagged = (margin < MARGIN_THRESH) | (winbase >= NPATCH)
    max_idx = np.empty(NPATCH, np.int64)

    # exact rescore of each row's winning 16-col segment, grouped by
    # segment so each group is one small sgemm
    order = np.argsort(winbase, kind="stable")
    ob = winbase[order]
    bounds = np.flatnonzero(np.diff(ob)) + 1
    for grp in np.split(order, bounds):
        b0 = winbase[grp[0]]
        if b0 >= NPATCH:
            continue
        cols = np.arange(b0, min(b0 + WINDOW, NPATCH))
        sc = inp_full[:, grp].T @ ref_full[:, cols]
        max_idx[grp] = cols[np.argmax(sc, axis=1)]
        if cols.size >= 2:
            t2 = np.partition(sc, cols.size - 2, axis=1)[:, -2:]
            risky = np.flatnonzero(t2[:, 1] - t2[:, 0] < 1e-3)
            if risky.size:
                sc64 = inp_full[:, grp[risky]].T.astype(np.float64) @ \
                    ref_full[:, cols].astype(np.float64)
                max_idx[grp[risky]] = cols[np.argmax(sc64, axis=1)]

    fl = np.flatnonzero(flagged)
    if fl.size:
        # exact rescore of near-tie rows over the full ref row: fp32 sgemm
        # first, fp64 only for rows still ambiguous at fp32 rounding scale
        corr = inp_full[:, fl].T @ ref_full
        max_idx[fl] = np.argmax(corr, axis=1)
        top2 = np.partition(corr, corr.shape[1] - 2, axis=1)[:, -2:]
        risky = np.flatnonzero(top2[:, 1] - top2[:, 0] < 1e-3)
        if risky.size:
            corr64 = inp_full[:, fl[risky]].T.astype(np.float64) @ \
                ref_full.astype(np.float64)
            max_idx[fl[risky]] = np.argmax(corr64, axis=1)
    return max_idx


def _flow_output(max_idx):
    # max_idx: (NPATCH,) int -> (18, H, W) float32, mirroring the reference
    mi = max_idx.reshape(HP, WP)
    fw = (mi % WP).astype(np.float32) - np.arange(WP, dtype=np.float32)[None, :]
    fh = (mi // WP).astype(np.float32) - np.arange(HP, dtype=np.float32)[:, None]
    flow = np.stack([fw, fh], axis=-1)                     # (94, 94, 2)
    flow = np.pad(flow, ((0, PS - 1), (0, PS - 1), (0, 0)))  # (96, 96, 2)
    shifted = np.stack([np.pad(flow, ((i, 0), (j, 0), (0, 0)))[:H, :W]
                        for i in range(PS) for j in range(PS)], axis=0)
    out = np.stack([shifted[..., 1], shifted[..., 0]], axis=1)  # (9, 2, H, W)
    return out.reshape(2 * PS * PS, H, W).astype(np.float32)


def kernel(dense_features1, dense_features2):
    from concourse import bass_utils

    nc = _get_nc()
    in_maps, mats = _prep_inputs(dense_features1, dense_features2)
    res = bass_utils.run_bass_kernel_spmd(
        nc, in_maps, core_ids=list(range(N_CORES)))
    out = np.empty((N_ITEMS, 2 * PS * PS, H, W), np.float32)
    for n in range(N_ITEMS):
        max_idx = _combine_item(
            [res.results[n * STRIPS + s] for s in range(STRIPS)], *mats[n])
        out[n] = _flow_output(max_idx)
    return out
